# revision 45
# baseline (speedup 1.0000x reference)
"""CrossModalTransformer Trainium2 kernel (8-core data parallel).

Strategy:
- Batch (8192) sharded across 8 NeuronCores (1024 each), processed in 8
  tiles of 128 batch elements (batch on the partition dim).
- Phase A (PE): convs + qkv projections in fp16 (1 cy/row matmuls) in
  feature-on-partition layout, then per-position PE transposes into
  batch-on-partition fp16 qkv buffers.
- Phase C (DVE/ACT/PE): head_dim=1 attention: scores are rank-1 outer
  products on DVE in packed-pair fp16 layouts (2x mode, duplicated-q
  tiles make every operand innermost-contiguous), exp on ScalarE in
  1024-el chunks, E and E*V interleaved per (head, q-group) in one
  combined bf16 tile so each PE Z/N-reduce matmul streams a 256-col
  rhs (half the matmul+ldweights count), o = N/Z.
- Phase 2 (PE): attention out-proj via fp16 block tables on
  PE-transposed chunks; LayerNorm in batch-on-partition layout
  (mean-subtraction folded into out-proj weights on the host).
- Phase 3: out-MHA over the 126-token concat, fc1, 3-way softmax.
"""
import os
import sys
import numpy as np

sys.path.insert(0, '/opt/trn_rl_repo')

import bass_rust
import concourse.bass as bass
import concourse.mybir as mybir
from concourse.tile import TileContext
from concourse.bass_utils import run_bass_kernel_spmd

FP = mybir.dt.float32
FR = mybir.dt.float32r
FH = mybir.dt.float16
BF = mybir.dt.bfloat16
AX = mybir.AxisListType
OP = mybir.AluOpType
AF = mybir.ActivationFunctionType

E = 8
NCORE = 8
B = 8192
BC = B // NCORE
P = 128
NBT = BC // P

L_E = 30
L_O = 32
MODS = ['e', 'p', 's', 'a', 'l']
LMOD = {'e': L_E, 'p': L_O, 's': L_O, 'a': L_O, 'l': L_O}
CROSS_OFF = {'e': 0, 'p': 30, 's': 62, 'a': 94, 'l': 126}
L_CROSS = 158
SELF_MODS = ['e', 'p', 'a']
SELF_OFF = {'e': 0, 'p': 30, 'a': 62}
L_SELF = 94
L_CAT = 126
CAT_OFF = {'e': 0, 'p': 30, 'a': 62, 's': 94}   # concat order: e, p, a, s

KV_GROUPS = {
    'e': ['p', 's', 'a'],
    'p': ['e', 'a', 's'],
    'a': ['e', 'p', 's'],
    'l': ['e', 'p', 's'],
    's': ['e', 'p', 'a'],
}
EPS = 1e-5
KEVPOOL = int(os.environ.get('KEVPOOL', '0'))  # EV multiply on gpsimd (outer)
KSCBUFS = int(os.environ.get('KSCBUFS', '1'))  # score tile double-buffering
KSPOOL = int(os.environ.get('KSPOOL', '20'))   # % of S-mults on gpsimd


def split_multi_waits(nc, max_waits=1):
    """This walrus build rejects >1 sem-wait on several instruction types:
    hoist extra waits onto NoOps inserted just before each instruction."""
    n = 0
    for fn in nc.m.functions:
        for bb in fn.blocks:
            insts = bb.instructions
            out = []
            changed = False
            for inst in insts:
                si = inst.sync_info
                waits = list(si.on_wait) if si is not None and si.on_wait else []
                if len(waits) > max_waits:
                    changed = True
                    n += 1
                    extra, keep = waits[:-max_waits], waits[-max_waits:]
                    for w in extra:
                        nop = bass_rust.InstNoOp(
                            name=f"waitsplit-{nc.next_id()}",
                            engine=inst.engine,
                            ins=[], outs=[],
                            sync_info=mybir.SyncInfo(on_wait=[w], on_update=[]),
                            bass_nofuse=True,
                        )
                        nc.register_instruction(nop, overwrite=True)
                        out.append(nop)
                    si.on_wait = keep
                    inst.sync_info = si
                out.append(inst)
            if changed:
                insts.clear()
                for i in out:
                    insts.append(i)
    return n


CONSTS_SPEC = {
    'w_eeg0': [40, 8], 'w_eeg1': [40, 8], 'w_psa': [2, 8],
    'w_loc': [3, 8], 'w_tgt': [1, 8],
    'bpe_e': [8, 1], 'bpe_psa': [8, 1], 'bpe_l': [8, 1], 'bpe_t': [8, 1],
    'w_cin': [9, 24], 'b_cin': [24, 1],
    'w_sin': [9, 24], 'b_sin': [24, 1],
    'w_oin': [9, 24], 'b_oin': [24, 1],
    'bo_k_o': [128, 1],
    'bk_o': [128, 1], 'bv_o': [128, 1],
    'gam_rep': [128, 8], 'bet_rep': [128, 8],
    'iden': [128, 128],
    'fc1_l0': [128, 90], 'fc1_l1': [128, 90], 'fc1_b': [90, 1],
    # k-major attend consts
    'bo_k_c': [128, 1], 'bo_k_s': [128, 1],
}
# consts loaded as fp16 (PE 1-cycle/row stationary operands)
CONSTS_FH = {'w_eeg0', 'w_eeg1', 'w_psa', 'w_loc', 'w_tgt',
             'w_cin', 'w_sin', 'w_oin'}
# bf16 consts (matmul partners of bf16 rhs)
CONSTS_BF = {
    'idenb': [128, 128],
}
# fp16 matmul tables (out-proj, out-kv-proj) + fp16 identity
CONSTS_FH2 = {
    'wblk_ko': [128, 128], 'wblk_vo': [128, 128],
    'idenh': [128, 128],
    'repq32': [32, 128],
}
# Z/N-reduce selection lhsT: onesZ_{qg}_{Lk} [128=(4j,32k), 32=(8qg',4j')]
for _qg in range(8):
    for _lk in (30, 32):
        CONSTS_BF[f'onesZ_{_qg}_{_lk}'] = [128, 32]
# outer-MHA Z/N selection lhsT: onesO_{q} [128=(128k), 32=(q')]
for _q in range(32):
    CONSTS_BF[f'onesO_{_q}'] = [128, 32]
# out-proj lhsT variants [64,128] fp16: (type, head-quarter, qhalf, Lq)
for _t in ('c', 's'):
    for _hq in range(4):
        for _qh in (0, 1):
            for _lq in (30, 32):
                CONSTS_FH2[f'wop_{_t}{_hq}{_qh}_{_lq}'] = [64, 128]
for _hq in range(4):
    for _qh in (0, 1):
        CONSTS_FH2[f'wop_o{_hq}{_qh}_32'] = [64, 128]


def build_program(reps=1, gamma_id=False, beta_id=False):
    nc = bass.Bass()

    def din(name, shape, dt=FP):
        return nc.declare_dram_parameter(name, list(shape), dt, isOutput=False)

    eeg_r = din("eeg_r", [40, BC, 118], FH)
    psa_r = din("psa_r", [2, 3, L_E, BC], FH)
    loc_r = din("loc_r", [3, L_E, BC], FH)
    tgt_r = din("tgt_r", [1, L_E, BC], FH)
    dparams = {}
    for k, v in CONSTS_SPEC.items():
        dparams[k] = din(k, v, FH if k in CONSTS_FH else FP)
    for k, v in CONSTS_BF.items():
        dparams[k] = nc.declare_dram_parameter(k, list(v), BF, isOutput=False)
    for k, v in CONSTS_FH2.items():
        dparams[k] = nc.declare_dram_parameter(k, list(v), FH, isOutput=False)
    out_d = nc.declare_dram_parameter("out", [BC, 90], FP, isOutput=True)

    from contextlib import ExitStack as _ES0
    with TileContext(nc) as tc:
        with _ES0() as _pstk:
            def _pool(name, bufs, space=None):
                kw = {"space": space} if space else {}
                return _pstk.enter_context(
                    tc.tile_pool(name=name, bufs=bufs, **kw))
            cpool = _pool("consts", 1)
            wp = _pool("wp", 1)
            iop = _pool("io", 2)
            qbp = _pool("qb", 2)
            nzp = _pool("nz", 2)
            obp = _pool("ob", 1)
            p2p = _pool("p2", 2)
            spool = _pool("sp", 2)
            etp = _pool("et", 2)
            vtp = _pool("vt", 4)
            otp = _pool("ot", 3)
            zrp = _pool("zr", 2)
            qrp = _pool("qr", 2)
            ppA = _pool("psA", 1, "PSUM")
            ppB = ppA
            ppT = _pool("psT", 1, "PSUM")
            ppK = _pool("pk", 2, "PSUM")
            ppZN = _pool("pzn", 2, "PSUM")
            pp2 = _pool("ps2", 2, "PSUM")

            C = {}
            for k, shp in CONSTS_SPEC.items():
                t = cpool.tile(list(shp), FH if k in CONSTS_FH else FP,
                               tag=k, name=f"c_{k}")
                nc.sync.dma_start(out=t[:], in_=dparams[k][:])
                C[k] = t
            for k, shp in CONSTS_BF.items():
                t = cpool.tile(list(shp), BF, tag=k, name=f"c_{k}")
                nc.sync.dma_start(out=t[:], in_=dparams[k][:])
                C[k] = t
            for k, shp in CONSTS_FH2.items():
                t = cpool.tile(list(shp), FH, tag=k, name=f"c_{k}")
                nc.sync.dma_start(out=t[:], in_=dparams[k][:])
                C[k] = t
            epsb = cpool.tile([128, 1], FP, tag="epsb", name="epsb")
            nc.vector.memset(epsb[:], EPS)
            tok_init = wp.tile([9, L_O * P], FH, tag="tok", name="tok_init")
            nc.vector.memset(tok_init[:], 1.0)

            bpe = {'e': C['bpe_e'], 'p': C['bpe_psa'], 's': C['bpe_psa'],
                   'a': C['bpe_psa'], 'l': C['bpe_l'], 't': C['bpe_t']}

            from contextlib import ExitStack as _ES
            with _ES() as _lc:
                if reps > 1:
                    _lc.enter_context(tc.For_i(0, reps, 1))
                for bt in range(NBT):
                    b0 = bt * P

                    # ============ Phase A ============
                    qb_c = qbp.tile([P, 24 * L_CROSS], FH, tag="qb_c")
                    qb_s = qbp.tile([P, 24 * L_SELF], FH, tag="qb_s")
                    qb_t = qbp.tile([P, 24 * L_O], FH, tag="qb_t")

                    def proj_transpose(tok, Lm, w, b_in, target, Ltot, off):
                        """tok [8,(Lm,128b)] --W--> [24,(Lm,128b)] --T-->
                        target [128b,(24ch,Ltot)] at L-offset off."""
                        qkv = wp.tile([24, Lm * P], FH, tag="qkv")
                        ncols = Lm * P
                        for c0 in range(0, ncols, 512):
                            cw = min(512, ncols - c0)
                            pj = ppA.tile([24, 512], FP, tag="pj")
                            nc.tensor.matmul(pj[:, 0:cw], w[:],
                                             tok[:, c0:c0 + cw],
                                             start=True, stop=True)
                            nc.vector.tensor_copy(qkv[:, c0:c0 + cw],
                                                  pj[:, 0:cw])
                        dst0 = target[:].rearrange("p (c l) -> p c l", c=24)
                        for L0 in range(0, Lm, 15):
                            Ln = min(15, Lm - L0)
                            tp = ppT.tile([P, 480], FP, tag="tp")
                            tpv = tp[:].bitcast(FH)[:, 0:480]
                            for Lx in range(Ln):
                                nc.tensor.transpose(
                                    tpv[:, Lx * 32:Lx * 32 + 24],
                                    qkv[:, (L0 + Lx) * P:(L0 + Lx + 1) * P],
                                    C['idenh'][0:24, 0:24])
                            src = tpv[:].rearrange(
                                "p (l s) -> p l s", s=32)[:, 0:Ln, 0:24]
                            src = src.transpose([0, 2, 1])      # [128, 24, Ln]
                            dst = dst0[:, :, off + L0:off + L0 + Ln]
                            nc.vector.tensor_copy(dst, src)

                    # --- eeg tokens: strided conv as 2 accumulated matmuls ---
                    tok_e = wp.tile([9, L_O * P], FH, tag="tok")
                    for sb in range(8):
                        bofs = b0 + sb * 16
                        chunk = iop.tile([40, 16 * 118], FH, tag="eegchunk")
                        nc.sync.dma_start(
                            out=chunk[:].rearrange("p (b w) -> p b w", b=16),
                            in_=eeg_r[:, bofs:bofs + 16, :])
                        base = chunk[:].rearrange("p (b w) -> p b w", b=16)
                        cvt = ppB.tile([8, 480], FP, tag="pj")
                        rh0 = base[:, :, 0:117:4].transpose([0, 2, 1])
                        rh1 = base[:, :, 1:118:4].transpose([0, 2, 1])
                        cout = cvt[:].rearrange("p (w b) -> p w b", b=16)
                        nc.tensor.matmul(cout, C['w_eeg0'][:], rh0,
                                         start=True, stop=False)
                        nc.tensor.matmul(cout, C['w_eeg1'][:], rh1,
                                         start=False, stop=True)
                        dste = tok_e[0:8, 0:L_E * P].rearrange("p (l b) -> p l b", b=P)
                        dste = dste[:, 0:30, sb * 16:(sb + 1) * 16]
                        nc.scalar.activation(dste, cout, AF.Identity,
                                             bias=bpe['e'][0:8, :],
                                             scale=1.0)
                    proj_transpose(tok_e, L_E, C['w_cin'], C['b_cin'],
                                   qb_c, L_CROSS, CROSS_OFF['e'])
                    proj_transpose(tok_e, L_E, C['w_sin'], C['b_sin'],
                                   qb_s, L_SELF, SELF_OFF['e'])

                    # --- conv_tgt-branch tokens (p, s, a, l, t) ---
                    def conv_k1(w, src_dram_ap, bpe_col):
                        tok = wp.tile([9, L_O * P], FH, tag="tok")
                        icn = src_dram_ap.shape[0]
                        chunk = iop.tile([4, L_E * P], FH, tag="k1chunk")
                        nc.sync.dma_start(
                            out=chunk[0:icn, :].rearrange("p (l b) -> p l b",
                                                          l=L_E),
                            in_=src_dram_ap)
                        ncols = L_E * P
                        for c0 in range(0, ncols, 480):
                            cw = min(480, ncols - c0)
                            cvt = ppB.tile([8, 480], FP, tag="pj")
                            nc.tensor.matmul(cvt[:, 0:cw], w[:],
                                             chunk[0:icn, c0:c0 + cw],
                                             start=True, stop=True)
                            nc.scalar.activation(
                                tok[0:8, P + c0:P + c0 + cw], cvt[:, 0:cw],
                                AF.Identity, bias=bpe_col[0:8, :], scale=1.0)
                        pad = tok[0:8, :].rearrange("p (l b) -> p l b", b=P)
                        pad = pad[:, 0:32:31, :]
                        nc.vector.tensor_scalar(
                            out=pad, in0=pad, scalar1=0.0, scalar2=bpe_col[:],
                            op0=OP.mult, op1=OP.add)
                        return tok

                    for i, mod in enumerate(['p', 's', 'a']):
                        tok = conv_k1(C['w_psa'], psa_r[:, i, :, b0:b0 + P],
                                      bpe[mod])
                        proj_transpose(tok, L_O, C['w_cin'], C['b_cin'],
                                       qb_c, L_CROSS, CROSS_OFF[mod])
                        if mod in SELF_MODS:
                            proj_transpose(tok, L_O, C['w_sin'], C['b_sin'],
                                           qb_s, L_SELF, SELF_OFF[mod])
                    tok = conv_k1(C['w_loc'], loc_r[:, :, b0:b0 + P], bpe['l'])
                    proj_transpose(tok, L_O, C['w_cin'], C['b_cin'],
                                   qb_c, L_CROSS, CROSS_OFF['l'])
                    tok = conv_k1(C['w_tgt'], tgt_r[:, :, b0:b0 + P], bpe['t'])
                    proj_transpose(tok, L_O, C['w_oin'], C['b_oin'], qb_t, L_O, 0)

                    # --- duplicated-q tiles (DVE 2x packed S-mult) ---
                    CROSS_Q = ['e', 'p', 's', 'a']

                    def build_qrep(buf, qmods, offmap, tag):
                        qr = qrp.tile([P, len(qmods) * 8 * 32 * 2], FH,
                                      tag=tag)
                        qr5 = qr[:].rearrange(
                            "p (m h q two) -> p m h q two",
                            m=len(qmods), h=8, two=2)
                        ch3 = buf[:].rearrange("p (c l) -> p c l", c=24)
                        for mi, m in enumerate(qmods):
                            Lq = LMOD[m]
                            nc.vector.tensor_copy(
                                qr5[:, mi, :, 0:Lq, :],
                                ch3[:, 0:8, offmap[m]:offmap[m] + Lq]
                                .unsqueeze(3).broadcast_to([P, 8, Lq, 2]))
                        return qr5

                    qrep_c = build_qrep(qb_c, CROSS_Q, CROSS_OFF, "qrc")
                    qrep_s = build_qrep(qb_s, SELF_MODS, SELF_OFF, "qrs")
                    qrep_t = build_qrep(qb_t, ['p'], {'p': 0}, "qrt")

                    # ============ Phase C1: 18 inner attentions (k-major) ====
                    _sctr = [0]

                    def build_vtile(buf, hh, off):
                        """v channels hh*4..+4 of buf, 32 k-slots at off ->
                        vt4 [128=(4j,32k), (4h,128b)] bf16 (v replicated on j)."""
                        ch3 = buf[:].rearrange("p (c l) -> p c l", c=24)
                        vt4 = vtp.tile([128, 512], BF, tag="vt4")
                        ps = ppK.tile([128, 1024], BF, tag="psT")
                        psv = ps[:].bitcast(FH)
                        for h in range(4):
                            nc.tensor.transpose(
                                psv[0:32, h * 128:h * 128 + 128],
                                ch3[:, 16 + hh * 4 + h, off:off + 32],
                                C['idenh'][:])
                        vs32 = vtp.tile([32, 512], FH, tag="vs32")
                        nc.scalar.copy(vs32[:], psv[0:32, 0:512])
                        vrep = ppK.tile([128, 512], FP, tag="psT")
                        nc.tensor.matmul(vrep[:], C['repq32'][:],
                                         vs32[:], start=True, stop=True)
                        nc.scalar.copy(vt4[:], vrep[:])
                        return vt4

                    _pend = []

                    def flush_pend():
                        while _pend:
                            _pend.pop(0)()

                    def attend_k(buf, offmap, qm, kvm, hh, vt4, qr5, mi):
                        """4 heads (hh half) of one (qmod,kvmod) ->
                        o_T [128=(4h,32q), 128b] fp32 SBUF."""
                        Lq, Lk = LMOD[qm], LMOD[kvm]
                        ch3 = buf[:].rearrange("p (c l) -> p c l", c=24)
                        S4 = spool.tile([P, 4096], BF, tag="S4")
                        S4v = S4[:].rearrange("p (h q k) -> p h q k",
                                              h=4, k=32)
                        S5v = S4[:].rearrange("p (h q k two) -> p h q k two",
                                              h=4, q=32, two=2)
                        if Lq < 32:
                            nc.gpsimd.memset(S4v[:, :, Lq:32, :], 0.0)
                        if Lk < 32:
                            nc.gpsimd.memset(S4v[:, :, 0:Lq, Lk:32], 0.0)
                        qrv = qr5[:, mi, hh * 4:hh * 4 + 4, :, :]
                        kvv = ch3[:, 8 + hh * 4:8 + hh * 4 + 4,
                                  offmap[kvm]:offmap[kvm] + Lk]
                        kv5 = kvv.rearrange("p c (k two) -> p c k two", two=2)
                        bsh2 = [P, Lq, Lk // 2, 2]
                        for h in range(4):
                            _sctr[0] += 1
                            eng = (nc.gpsimd if (_sctr[0] % 100) < KSPOOL
                                   else nc.vector)
                            eng.tensor_tensor(
                                out=S5v[:, h, 0:Lq, 0:Lk // 2, :],
                                in0=qrv[:, h, 0:Lq, :]
                                .unsqueeze(2).broadcast_to(bsh2),
                                in1=kv5[:, h, :, :]
                                .unsqueeze(1).broadcast_to(bsh2),
                                op=OP.mult)
                            if h == 1:
                                flush_pend()
                        ETEV = etp.tile([P, 8192], BF, tag="ET")
                        E5 = ETEV[:].rearrange(
                            "p (h qg t b) -> p h qg t b", h=4, qg=8, t=2)
                        for h in range(4):
                            ps = ppK.tile([128, 1024], BF, tag="psT")
                            for qg8 in range(8):
                                c0 = h * 1024 + qg8 * 128
                                nc.tensor.transpose(
                                    ps[:, qg8 * 128:qg8 * 128 + 128],
                                    S4[:, c0:c0 + 128], C['idenb'][:])
                            nc.scalar.activation(
                                E5[:, h, :, 0, :], ps[:], AF.Exp)
                        v4 = vt4[:].rearrange("p (h b) -> p h b", h=4)
                        oTs = []
                        for sub in (0, 1):
                            nc.vector.tensor_tensor(
                                out=E5[:, sub * 2:sub * 2 + 2, :, 1, :],
                                in0=E5[:, sub * 2:sub * 2 + 2, :, 0, :],
                                in1=v4[:, sub * 2:sub * 2 + 2, :]
                                .unsqueeze(2).broadcast_to([P, 2, 8, P]),
                                op=OP.mult)
                            ZN = ppZN.tile([64, 256], FP, tag="ZN")
                            for h in (sub * 2, sub * 2 + 1):
                                r0 = (h % 2) * 32
                                for qg in range(8):
                                    nc.tensor.matmul(
                                        ZN[r0:r0 + 32, 0:256],
                                        C[f'onesZ_{qg}_{Lk}'][:],
                                        E5[:, h, qg, :, :],
                                        start=(qg == 0), stop=(qg == 7))
                            oT = otp.tile([64, 128], FH,
                                          tag=f"oT{hh * 2 + sub}",
                                          name=f"oT{hh * 2 + sub}")

                            def fin(ZN=ZN, oT=oT):
                                Zr = zrp.tile([64, 128], FP, tag="Zr")
                                nc.vector.reciprocal(
                                    Zr[:], ZN[:, 0:128])
                                nc.vector.tensor_tensor(out=oT[:],
                                                        in0=ZN[:, 128:256],
                                                        in1=Zr[:],
                                                        op=OP.mult)
                            _pend.append(fin)
                            oTs.append(oT)
                        return oTs

                    o_bufs = {}

                    def outproj_k(key, quarters, Lq, t, bkey):
                        obt = obp.tile([P, 256], FP, tag=f"o_{key[0]}_{key[1]}",
                                       name=f"o_{key[0]}_{key[1]}")
                        for qh in (0, 1):
                            o2 = pp2.tile([128, 128], FP, tag="pps")
                            for hq in range(4):
                                nc.tensor.matmul(
                                    o2[:], C[f'wop_{t}{hq}{qh}_{Lq}'][:],
                                    quarters[hq][:],
                                    start=(hq == 0), stop=(hq == 3))
                            o2s = p2p.tile([128, 128], FH, tag="s1")
                            nc.scalar.add(o2s[:], o2[:], C[bkey][:])
                            tb = pp2.tile([128, 128], FP, tag="pps")
                            tbv = tb[:].bitcast(FH)[:, 0:128]
                            nc.tensor.transpose(tbv, o2s[:], C['idenh'][:])
                            nc.scalar.copy(obt[:, qh * 128:qh * 128 + 128],
                                           tbv)
                        o_bufs[key] = obt

                    def full_attend(buf, offmap, qm, kvm, vts, t, bkey,
                                    qr5, mi):
                        qt = (attend_k(buf, offmap, qm, kvm, 0, vts[0],
                                       qr5, mi)
                              + attend_k(buf, offmap, qm, kvm, 1, vts[1],
                                         qr5, mi))
                        _pend.append(
                            lambda: outproj_k((qm, kvm), qt, LMOD[qm],
                                              t, bkey))

                    for kv in MODS:
                        vts = [build_vtile(qb_c, hh, CROSS_OFF[kv])
                               for hh in (0, 1)]
                        for qm in KV_GROUPS[kv]:
                            full_attend(qb_c, CROSS_OFF, qm, kv, vts,
                                        'c', 'bo_k_c', qrep_c,
                                        CROSS_Q.index(qm))
                    for m in SELF_MODS:
                        vts = [build_vtile(qb_s, hh, SELF_OFF[m])
                               for hh in (0, 1)]
                        full_attend(qb_s, SELF_OFF, m, m, vts,
                                    's', 'bo_k_s', qrep_s,
                                    SELF_MODS.index(m))

                    # ============ Phase 2: out-proj + LN + concat ============
                    cat = obp.tile([P, L_CAT * 8], FP, tag="cat")
                    cat_first = {m: True for m in CAT_OFF}

                    var_all = obp.tile([P, 18 * 32], FP, tag="var_all")
                    inv_all = obp.tile([P, 18 * 32], FP, tag="inv_all")

                    def out_proj_part1(ob, Lq, wkey, bkey, mi):
                        nq = Lq * 8
                        sq = p2p.tile([P, L_O * 8], FP, tag="sq")
                        nc.scalar.activation(sq[:, 0:nq], ob[:, 0:nq],
                                             AF.Square)
                        nc.vector.tensor_reduce(
                            out=var_all[:, mi * 32:mi * 32 + Lq],
                            in_=sq[:, 0:nq].rearrange("p (q c) -> p q c", c=8),
                            axis=AX.X, op=OP.add)

                    def out_proj_part2(ob, Lq, mi, targets):
                        nq = Lq * 8
                        inv = inv_all[:, mi * 32:mi * 32 + Lq]
                        x3 = ob[:, 0:nq].rearrange("p (q c) -> p q c", c=8)
                        nc.vector.tensor_tensor(
                            out=x3, in0=x3,
                            in1=inv.unsqueeze(2).broadcast_to([P, Lq, 8]),
                            op=OP.mult)
                        if not gamma_id:
                            nc.vector.tensor_tensor(
                                out=x3, in0=x3,
                                in1=C['gam_rep'][:].unsqueeze(1).broadcast_to(
                                    [P, Lq, 8]),
                                op=OP.mult)
                        if not beta_id:
                            nc.vector.tensor_tensor(
                                out=x3, in0=x3,
                                in1=C['bet_rep'][:].unsqueeze(1).broadcast_to(
                                    [P, Lq, 8]),
                                op=OP.add)
                        for tmod in targets:
                            coff = CAT_OFF[tmod] * 8
                            cslice = cat[:, coff:coff + nq]
                            if cat_first[tmod]:
                                nc.vector.tensor_copy(cslice, ob[:, 0:nq])
                                cat_first[tmod] = False
                            else:
                                nc.vector.tensor_tensor(
                                    out=cslice, in0=cslice, in1=ob[:, 0:nq],
                                    op=OP.add)

                    mha_list = []
                    for kv in MODS:
                        for qm in KV_GROUPS[kv]:
                            targets = [qm] if qm in CAT_OFF else []
                            if (qm, kv) == ('s', 'l'):
                                targets.append('a')   # reference's reused term
                            mha_list.append(((qm, kv), LMOD[qm],
                                             'wblk_co', 'bo_co', targets))
                    for m in SELF_MODS:
                        mha_list.append(((m, m), LMOD[m],
                                         'wblk_so', 'bo_so', [m]))
                    flush_pend()
                    for mi, (key, Lq, wk, bk, tg) in enumerate(mha_list):
                        out_proj_part1(o_bufs[key], Lq, wk, bk, mi)
                    sig_all = obp.tile([P, 18 * 32], FP, tag="sig_all")
                    nc.scalar.activation(sig_all[:], var_all[:], AF.Sqrt,
                                         bias=epsb[0:P, :], scale=0.125)
                    nc.vector.reciprocal(inv_all[:], sig_all[:])
                    for mi, (key, Lq, wk, bk, tg) in enumerate(mha_list):
                        out_proj_part2(o_bufs[key], Lq, mi, tg)

                    # kv-projection of concat under out_in_w
                    k_out = obp.tile([P, 8 * L_CAT], FH, tag="k_out")
                    v_out = obp.tile([P, 8 * L_CAT], FH, tag="v_out")
                    for L0 in range(0, L_CAT, 16):
                        Ln = min(16, L_CAT - L0)
                        cw = Ln * 8
                        t1 = pp2.tile([128, 128], FP, tag="pps")
                        nc.tensor.transpose(t1[0:cw, :],
                                            cat[:, L0 * 8:L0 * 8 + cw],
                                            C['iden'][:])
                        s1 = p2p.tile([128, 128], FH, tag="s1")
                        nc.scalar.copy(s1[0:cw, :], t1[0:cw, :])
                        for wkey, bkey, target in [('wblk_ko', 'bk_o', k_out),
                                                   ('wblk_vo', 'bv_o', v_out)]:
                            m2 = pp2.tile([128, 128], FP, tag="pps")
                            nc.tensor.matmul(m2[0:cw, :],
                                             C[wkey][0:cw, 0:cw], s1[0:cw, :],
                                             start=True, stop=True)
                            s2 = p2p.tile([128, 128], FH, tag="s2")
                            nc.scalar.add(s2[0:cw, :], m2[0:cw, :],
                                          C[bkey][0:cw, :])
                            t2 = pp2.tile([128, 128], FP, tag="pps")
                            t2v = t2[:].bitcast(FH)[:, 0:128]
                            nc.tensor.transpose(t2v[:, 0:cw], s2[0:cw, :],
                                                C['idenh'][0:cw, 0:cw])
                            src = t2v[:, 0:cw].rearrange("p (l h) -> p l h", h=8)
                            dst = target[:].rearrange("p (h l) -> p h l", h=8)
                            dst = dst[:, :, L0:L0 + Ln].transpose([0, 2, 1])
                            nc.scalar.copy(dst, src)

                    # ============ Phase C2: out-MHA (k-major) ============
                    kv3 = k_out[:].rearrange("p (h l) -> p h l", h=8)
                    qt3 = qb_t[:].rearrange("p (h l) -> p h l", h=24)
                    oq = []
                    for hp in range(4):
                        ZNo = ppZN.tile([64, 256], FP, tag="ZN")
                        for hl in (0, 1):
                            h = hp * 2 + hl
                            vs_ps = pp2.tile([128, 128], FP, tag="pps")
                            vs_psv = vs_ps[:].bitcast(FH)[:, 0:128]
                            nc.tensor.transpose(
                                vs_psv[0:126, :],
                                v_out[:, h * L_CAT:h * L_CAT + L_CAT],
                                C['idenh'][:])
                            vs = vtp.tile([128, 128], BF, tag="vs")
                            nc.vector.memset(vs[64:128, :], 0.0)
                            nc.scalar.copy(vs[0:126, :], vs_psv[0:126, 0:128])
                            S4 = spool.tile([P, 4096], BF, tag="S4")
                            S4v = S4[:].rearrange("p (q k) -> p q k", k=128)
                            S5o = S4[:].rearrange(
                                "p (q k two) -> p q k two", q=32, two=2)
                            nc.gpsimd.memset(S4v[:, :, 126:128], 0.0)
                            kv5o = kv3[:, h, :].rearrange(
                                "p (k two) -> p k two", two=2)
                            bshape = [P, 32, L_CAT // 2, 2]
                            _sctr[0] += 1
                            eng = (nc.gpsimd if (_sctr[0] % 100) < KSPOOL
                                   else nc.vector)
                            eng.tensor_tensor(
                                out=S5o[:, :, 0:L_CAT // 2, :],
                                in0=qrep_t[:, 0, h, :, :].unsqueeze(2)
                                .broadcast_to(bshape),
                                in1=kv5o.unsqueeze(1)
                                .broadcast_to(bshape),
                                op=OP.mult)
                            flush_pend()
                            ETEV = etp.tile([P, 8192], BF, tag="ET")
                            E5o = ETEV[:].rearrange(
                                "p (q t b) -> p q t b", q=32, t=2)
                            for g in range(4):
                                ps = ppK.tile([128, 1024], BF, tag="psT")
                                for qq in range(8):
                                    c0 = g * 1024 + qq * 128
                                    nc.tensor.transpose(
                                        ps[:, qq * 128:qq * 128 + 128],
                                        S4[:, c0:c0 + 128], C['idenb'][:])
                                nc.scalar.activation(
                                    E5o[:, g * 8:g * 8 + 8, 0, :], ps[:],
                                    AF.Exp)
                            for qh2 in (0, 1):
                                nc.vector.tensor_tensor(
                                    out=E5o[:, qh2 * 16:qh2 * 16 + 16, 1, :],
                                    in0=E5o[:, qh2 * 16:qh2 * 16 + 16, 0, :],
                                    in1=vs[:].unsqueeze(1)
                                    .broadcast_to([P, 16, P]),
                                    op=OP.mult)
                            for q in range(32):
                                nc.tensor.matmul(
                                    ZNo[hl * 32:hl * 32 + 32, 0:256],
                                    C[f'onesO_{q}'][:],
                                    E5o[:, q, :, :],
                                    start=(q == 0), stop=(q == 31))
                        oTq = otp.tile([64, 128], FH, tag=f"oTo{hp}",
                                       name=f"oTo{hp}")

                        def fin_o(ZNo=ZNo, oTq=oTq):
                            Zr = zrp.tile([64, 128], FP, tag="Zr")
                            nc.vector.reciprocal(
                                Zr[:], ZNo[:, 0:128])
                            nc.vector.tensor_tensor(out=oTq[:],
                                                    in0=ZNo[:, 128:256],
                                                    in1=Zr[:], op=OP.mult)
                        _pend.append(fin_o)
                        oq.append(oTq)
                    flush_pend()

                    # ============ Phase 3: out-proj, fc1, softmax ============
                    rtiles = []
                    for qh in (0, 1):
                        o2 = pp2.tile([128, 128], FP, tag="pps")
                        for hq in range(4):
                            nc.tensor.matmul(
                                o2[:], C[f'wop_o{hq}{qh}_32'][:], oq[hq][:],
                                start=(hq == 0), stop=(hq == 3))
                        s2 = p2p.tile([128, 128], FP, tag=f"r{16 * qh}")
                        nc.scalar.add(s2[:], o2[:], C['bo_k_o'][:])
                        rtiles.append(s2)
                    fcp = pp2.tile([90, 128], FP, tag="pps")
                    nc.tensor.matmul(fcp[:], C['fc1_l0'][:], rtiles[0][:],
                                     start=True, stop=False)
                    nc.tensor.matmul(fcp[:], C['fc1_l1'][:], rtiles[1][:],
                                     start=False, stop=True)
                    sbf = p2p.tile([90, 128], FP, tag="sbf")
                    nc.scalar.add(sbf[:], fcp[:], C['fc1_b'][:])
                    ftp = pp2.tile([128, 90], FP, tag="pps")
                    nc.tensor.transpose(ftp[:], sbf[:], C['iden'][0:90, 0:90])
                    lg = p2p.tile([128, 90], FP, tag="lg")
                    nc.scalar.activation(lg[:], ftp[:], AF.Exp)
                    sm = nzp.tile([P, 32], FP, tag="sm")
                    nc.vector.tensor_reduce(
                        out=sm[:, 0:30],
                        in_=lg[:].rearrange("p (l c) -> p l c", c=3),
                        axis=AX.X, op=OP.add)
                    smr = nzp.tile([P, 32], FP, tag="smr")
                    nc.vector.reciprocal(smr[:, 0:30], sm[:, 0:30])
                    prob = p2p.tile([128, 90], FP, tag="prob")
                    nc.vector.tensor_tensor(
                        out=prob[:].rearrange("p (l c) -> p l c", c=3),
                        in0=lg[:].rearrange("p (l c) -> p l c", c=3),
                        in1=smr[:, 0:30].unsqueeze(2).broadcast_to([P, 30, 3]),
                        op=OP.mult)
                    nc.sync.dma_start(out=out_d[b0:b0 + P, :], in_=prob[:])

    split_multi_waits(nc)
    return nc


def pe_row(pos, d=E):
    i = np.arange(0, d, 2, dtype=np.float32)
    div = np.exp(i * (-np.log(10000.0) / d))
    row = np.zeros((d,), np.float32)
    row[0::2] = np.sin(pos * div)
    row[1::2] = np.cos(pos * div)
    return row


def host_consts(inp):
    IM = np.eye(8, dtype=np.float64) - np.full((8, 8), 0.125, np.float64)
    pe30 = pe_row(30.0)
    pe32 = pe_row(32.0)
    f32 = np.float32
    f16 = np.float16
    c = {}
    c['w_eeg0'] = np.ascontiguousarray(
        inp['eeg_conv_w'][:, :, :, 0].reshape(8, 40).T).astype(f16)
    c['w_eeg1'] = np.ascontiguousarray(
        inp['eeg_conv_w'][:, :, :, 1].reshape(8, 40).T).astype(f16)
    c['w_psa'] = np.ascontiguousarray(inp['psa_conv_w'][:, :, 0].T).astype(f16)
    c['w_loc'] = np.ascontiguousarray(inp['loc_conv_w'][:, :, 0].T).astype(f16)
    c['w_tgt'] = np.ascontiguousarray(inp['tgt_conv_w'][:, :, 0].T).astype(f16)
    c['bpe_e'] = (inp['eeg_conv_b'] + pe30).reshape(8, 1).astype(f32)
    c['bpe_psa'] = (inp['psa_conv_b'] + pe32).reshape(8, 1).astype(f32)
    c['bpe_l'] = (inp['loc_conv_b'] + pe32).reshape(8, 1).astype(f32)
    c['bpe_t'] = (inp['tgt_conv_b'] + pe32).reshape(8, 1).astype(f32)
    c['w_cin'] = np.vstack([inp['cross_in_w'].T,
                            inp['cross_in_b'][None, :]]).astype(f16)
    c['b_cin'] = inp['cross_in_b'].reshape(24, 1).astype(f32)
    c['w_sin'] = np.vstack([inp['self_in_w'].T,
                            inp['self_in_b'][None, :]]).astype(f16)
    c['b_sin'] = inp['self_in_b'].reshape(24, 1).astype(f32)
    c['w_oin'] = np.vstack([inp['out_in_w'].T,
                            inp['out_in_b'][None, :]]).astype(f16)
    c['b_oin'] = inp['out_in_b'].reshape(24, 1).astype(f32)
    import ml_dtypes
    bf = ml_dtypes.bfloat16
    I16 = np.eye(16)
    co = IM @ inp['cross_out_w'].astype(np.float64)
    so = IM @ inp['self_out_w'].astype(np.float64)
    c['bo_k_o'] = np.tile(inp['out_out_b'], 16).reshape(128, 1).astype(f32)
    c['bo_k_c'] = np.tile(IM @ inp['cross_out_b'], 16).reshape(128, 1).astype(f32)
    c['bo_k_s'] = np.tile(IM @ inp['self_out_b'], 16).reshape(128, 1).astype(f32)
    # k-major out-proj lhsT: wop[(2h,32q),(16q',8co)] =
    #   (q == qh*16+q') * W[co, hq*2+h], zeroed at q >= Lq
    for t, wmat in (('c', co), ('s', so),
                    ('o', inp['out_out_w'].astype(np.float64))):
        for hq in range(4):
            for qh in (0, 1):
                for lq in (30, 32):
                    w = np.zeros((2, 32, 16, 8), np.float64)
                    for h in range(2):
                        for q in range(32):
                            if q >= lq or (q // 16) != qh:
                                continue
                            w[h, q, q % 16, :] = wmat[:, hq * 2 + h]
                    c[f'wop_{t}{hq}{qh}_{lq}'] = (
                        w.reshape(64, 128).astype(np.float16))
    for qg in range(8):
        for lk in (30, 32):
            o4 = np.zeros((4, 32, 8, 4), np.float32)   # [(j,k),(qg',j')]
            for j in range(4):
                o4[j, 0:lk, qg, j] = 1.0
            c[f'onesZ_{qg}_{lk}'] = o4.reshape(128, 32).astype(bf)
    r = np.zeros((32, 4, 32), np.float32)          # [k,(j,k')]
    for k in range(32):
        r[k, :, k] = 1.0
    c['repq32'] = r.reshape(32, 128).astype(f16)
    for q in range(32):
        oo = np.zeros((128, 32), np.float32)
        oo[0:126, q] = 1.0
        c[f'onesO_{q}'] = oo.astype(bf)
    c['idenb'] = np.eye(128, dtype=np.float32).astype(bf)
    c['idenh'] = np.eye(128, dtype=np.float16)
    c['wblk_ko'] = np.kron(I16, inp['out_in_w'][8:16].T).astype(f16)
    c['wblk_vo'] = np.kron(I16, inp['out_in_w'][16:24].T).astype(f16)
    c['bk_o'] = np.tile(inp['out_in_b'][8:16], 16).reshape(128, 1).astype(f32)
    c['bv_o'] = np.tile(inp['out_in_b'][16:24], 16).reshape(128, 1).astype(f32)
    c['gam_rep'] = np.tile(inp['norm_g'], (128, 1)).astype(f32)
    c['bet_rep'] = np.tile(inp['norm_b'], (128, 1)).astype(f32)
    c['iden'] = np.eye(128, dtype=f32)
    fc1T = np.ascontiguousarray(inp['fc1_w'].astype(f32).T)   # [256, 90]
    c['fc1_l0'] = np.ascontiguousarray(fc1T[0:128])
    c['fc1_l1'] = np.ascontiguousarray(fc1T[128:256])
    c['fc1_b'] = inp['fc1_b'].reshape(90, 1).astype(f32)
    return c


_PROG_CACHE = {}


def prep_in_maps(inputs):
    consts = host_consts(inputs)
    f16 = np.float16
    eeg = np.asarray(inputs['eeg'], dtype=f16)
    eeg_r_all = np.ascontiguousarray(
        eeg.reshape(B, 40, 118).transpose(1, 0, 2))          # [40, B, 118]
    psa_all = np.ascontiguousarray(
        np.stack([np.asarray(inputs['pupil'], f16),
                  np.asarray(inputs['speech'], f16),
                  np.asarray(inputs['action'], f16)], 0)
        .transpose(2, 0, 3, 1))                              # [2, 3, 30, B]
    loc_all = np.ascontiguousarray(
        np.asarray(inputs['location'], f16).transpose(1, 2, 0))  # [3, 30, B]
    tgt_all = np.ascontiguousarray(
        np.asarray(inputs['tgt'], f16).T[None, :, :])        # [1, 30, B]

    in_maps = []
    for core in range(NCORE):
        s = slice(core * BC, (core + 1) * BC)
        m = dict(consts)
        m['eeg_r'] = np.ascontiguousarray(eeg_r_all[:, s, :])
        m['psa_r'] = np.ascontiguousarray(psa_all[:, :, :, s])
        m['loc_r'] = np.ascontiguousarray(loc_all[:, :, s])
        m['tgt_r'] = np.ascontiguousarray(tgt_all[:, :, s])
        in_maps.append(m)
    return in_maps


def kernel(**inputs):
    gid = bool(np.all(np.asarray(inputs['norm_g']) == 1.0))
    bid = bool(np.all(np.asarray(inputs['norm_b']) == 0.0))
    key = ('nc', gid, bid)
    if key not in _PROG_CACHE:
        _PROG_CACHE[key] = build_program(gamma_id=gid, beta_id=bid)
    nc = _PROG_CACHE[key]
    in_maps = prep_in_maps(inputs)
    res = run_bass_kernel_spmd(nc, in_maps, list(range(NCORE)))
    outs = [res.results[i]["out"] for i in range(NCORE)]
    full = np.concatenate(outs, axis=0)                       # [B, 90]
    return np.ascontiguousarray(
        full.reshape(B, 30, 3).transpose(0, 2, 1)).astype(np.float32)

